# revision 36
# baseline (speedup 1.0000x reference)
"""Trainium2 Bass kernel for nn_CMA_Block (cross-modal attention block).

Per-sample pipeline (data-parallel over B=8 across 8 NeuronCores):
  rgb(bf16),freq(fp8) -> avgpool2 -> QKV 1x1-conv projections (pool folded
  into accumulating matmuls; q/k quantized to fp8 with power-of-2 scales;
  q-bias folded into a 65th k-row via wk2 = [wk | wk@qb]) ->
  S^T = K^T Q via fp8 DoubleRow matmuls (stride-0 broadcast pair dims;
  exp-scale folded into the activation scale) -> exp on ACT/DVE -> fp8
  DoubleRow AV with V' = Wo'V pre-folded (conv1x1 eliminated; ones channel
  gives the softmax denominator) -> normalize + 2x bilinear upsample as a
  prescale/strided-add chain on DVE fast modes -> LeakyReLU -> residual add
  on GPSIMD -> bf16 output DMA (host converts to f32).
"""

import sys

sys.path.insert(0, "/opt/trn_rl_repo")

import numpy as np
import ml_dtypes

import concourse.bass as bass
import concourse.bacc as bacc
import concourse.mybir as mybir
import concourse.tile as tile
from concourse.bass_utils import run_bass_kernel_spmd
import concourse.dve_ops as dve_ops
from concourse.dve_spec import (
    Spec, Src0, C0, C1, C2, sq, lower, _has_src1 as has_src1,
)
from concourse.dve_uop import DveOpSpec

# exp(x) ~= ((EC2*x + EC1)*x + EC0)^16, max rel err 5.5e-4 on [-1.5, 1.5]
EC0, EC1, EC2 = 1.0000024, 0.06256861, 0.00195205


def _register_exp_op():
    """Register a one-pass DVE polynomial exp (quadratic seed + 4 squarings)."""
    name = "EXP_POLY16_ANT"
    for op in dve_ops.OPS:
        if op.name == name:
            return op
    body = sq(sq(sq(sq((Src0 * C2 + C1) * Src0 + C0))))
    spec = Spec(
        body=body,
        reference=lambda in0, in1, s0, s1, imm2: (
            (((in0 * imm2 + s1) * in0 + s0)) ** 16
        ).astype(np.float32),
    )
    row = dve_ops._CUSTOM_DVE_ROW_BASE + len(dve_ops.OPS)
    dve_ops._SUB_OPCODE_FOR_NAME[name] = row
    shas = {}
    for ver in ("v3", "v4"):
        sp = DveOpSpec(
            name=name, opcode=row, uops=lower(spec, ver=ver),
            rd1_en=has_src1(spec),
        )
        shas[ver] = sp.sha(ver)
    op = dve_ops.DveOp(name, spec, subdim=False, uops_sha=shas)
    dve_ops.OPS.append(op)
    dve_ops.CUSTOM_DVE_SPECS[name] = spec
    return op


EXP_OP = _register_exp_op()

F32 = mybir.dt.float32
F32R = mybir.dt.float32r
BF16 = mybir.dt.bfloat16
FP8 = mybir.dt.float8e4
AF = mybir.ActivationFunctionType
ALU = mybir.AluOpType
DR = mybir.MatmulPerfMode.DoubleRow

# Problem shape constants (hardcoded per contract).
B = 8          # batch == n_cores
C = 64         # channels (Cin == Hid == Cout == 64)
H = 128        # full-res H == W
HW = H * H     # 16384
HD = 64        # pooled H == W
N = HD * HD    # 4096 tokens
NB = 8         # n-blocks of 512 tokens
BLK = N // NB  # 512
MT = 32        # m-tiles of 128 tokens
NP = 4         # block pairs (1024 tokens each)
NEG_SLOPE = 0.2
BN_EPS = 1e-5

# fp8 scale plan: q8 = AQ*q_raw, k8 = AK*k_raw; stride-0 DoubleRow doubles
# the product; exp() folds sigma = attn_scale / (AQ*AK*2) back in.
AQ = 4.0
AK = 4.0
SIG = (C ** -0.5) / (AQ * AK * 2.0)   # 2^-8
SV = 4.0                              # V'8 = SV * Wo' V
SVC = 0.5625 / SV                     # stt scalar: 0.5625/SV

# exp engine split: each [128,1024] tile is split column-wise, ACT takes
# the first ECOLS columns, DVE (custom poly op) the rest — both engines
# run in lockstep on every tile.
ECOLS = 640


def build_program(debug=False, taps=False):
    """Build the per-core (SPMD) bass program. Returns (nc, io_names)."""
    nc = bacc.Bacc(
        "TRN2",
        target_bir_lowering=False,
        debug=debug,
        enable_asserts=False,
        num_devices=B,
    )

    # DRAM I/O (per-core slices of the batch; weights replicated).
    rgb_d = nc.dram_tensor("rgb", [C, HW], BF16, kind="ExternalInput").ap()
    freq_d = nc.dram_tensor("freq", [C, HW], FP8, kind="ExternalInput").ap()
    wq_d = nc.dram_tensor("wq_l", [C, C], BF16, kind="ExternalInput").ap()
    wk_d = nc.dram_tensor("wk_l", [C + 1, C], BF16, kind="ExternalInput").ap()
    wv_d = nc.dram_tensor("wv2", [C + 1, C], BF16, kind="ExternalInput").ap()
    wi_d = nc.dram_tensor("wi_l", [2 * C, C], FP8, kind="ExternalInput").ap()
    b75_d = nc.dram_tensor("b75", [C, 1], F32, kind="ExternalInput").ap()
    onesb_d = nc.dram_tensor("onesb", [1, N], BF16, kind="ExternalInput").ap()
    out_d = nc.dram_tensor("out", [C, HW], BF16, kind="ExternalOutput").ap()
    recd = nc.dram_tensor("rec_scratch", [NB, BLK], F32).ap()
    if taps:
        fds_o = nc.dram_tensor("fds_o", [C + 1, N], BF16, kind="ExternalOutput").ap()
        qd_o = nc.dram_tensor("qd_o", [C, N], FP8, kind="ExternalOutput").ap()
        kd_o = nc.dram_tensor("kd_o", [C, N], FP8, kind="ExternalOutput").ap()
        vt_o = nc.dram_tensor("vt_o", [128, MT * C], FP8,
                              kind="ExternalOutput").ap()
        av_o = nc.dram_tensor("av_o", [C + 1, N], F32, kind="ExternalOutput").ap()
        t56_o = nc.dram_tensor("t56_o", [C, N], BF16, kind="ExternalOutput").ap()
        xup_o = nc.dram_tensor("xup_o", [C, 2 * N], BF16, kind="ExternalOutput").ap()
        v_o = nc.dram_tensor("v_o", [C, HW], BF16, kind="ExternalOutput").ap()

    with tile.TileContext(nc) as tc:
        with (
            tc.tile_pool(name="const", bufs=1) as cpool,
            tc.tile_pool(name="persist", bufs=1) as perm,
        ):
            # ---- constants ----
            wq_t = cpool.tile([C, C], BF16, tag="wq")
            wk_t = cpool.tile([C + 1, C], BF16, tag="wk")
            wv_t = cpool.tile([C + 1, C], BF16, tag="wv")
            wi_t = cpool.tile([2 * C, C], FP8, tag="wi")
            b75_t = cpool.tile([C, 1], F32, tag="b75")
            nc.sync.dma_start(wq_t[:], wq_d)
            nc.sync.dma_start(wk_t[:], wk_d)
            nc.sync.dma_start(wv_t[:], wv_d)
            nc.sync.dma_start(wi_t[:], wi_d)
            nc.sync.dma_start(b75_t[:], b75_d)

            # ---- persistent SBUF tensors ----
            rgb_t = perm.tile([C, HW], BF16, tag="rgb")      # Q rhs + residual
            fds_t = perm.tile([C + 1, N], BF16, tag="fds")   # pooled freq +ones
            qd8_t = perm.tile([C, N], FP8, tag="qd8")        # q8 fp8
            kd8_t = perm.tile([C, N], FP8, tag="kd8")        # k8 fp8
            vt8_t = perm.tile([128, MT * C], FP8, tag="vt8")  # V'8^T tiles
            one8_t = perm.tile([128, 128], FP8, tag="one8")  # DR den-dup lhsT

            for p in range(NP):
                sl = slice(p * 4096, (p + 1) * 4096)
                nc.sync.dma_start(rgb_t[:, sl], rgb_d[:, sl])
            nc.gpsimd.dma_start(fds_t[C : C + 1, :], onesb_d)

            with (
                tc.tile_pool(name="p1sb", bufs=1) as p1sb,
                tc.tile_pool(name="pp1", bufs=2, space="PSUM") as pp1,
                tc.tile_pool(name="ppv", bufs=2, space="PSUM") as ppv,
            ):
                freq_t = p1sb.tile([C, HW], FP8, tag="freq")
                for p in range(NP):
                    sl = slice(p * 4096, (p + 1) * 4096)
                    nc.scalar.dma_start(freq_t[:, sl], freq_d[:, sl])

                # ---- phase 1a: pool freq via 4 accumulating fp8 matmuls ----
                freq_r = freq_t[:].rearrange(
                    "p (r a x c) -> p r a x c", r=HD, a=2, x=HD, c=2
                )
                for b in range(NB):
                    sl = slice(b * BLK, (b + 1) * BLK)
                    psf = pp1.tile([C, BLK], F32, tag="psf")
                    k = 0
                    for dy in range(2):
                        for dx in range(2):
                            nc.tensor.matmul(
                                psf[:],
                                wi_t[0:C, :],
                                freq_r[:, 8 * b : 8 * b + 8, dy, :, dx],
                                start=(k == 0),
                                stop=(k == 3),
                            )
                            k += 1
                    nc.vector.tensor_copy(fds_t[0:C, sl], psf[:])

                # ---- phase 1b: K (wk2 includes q-bias row as output 64) ----
                for b in range(NB):
                    sl = slice(b * BLK, (b + 1) * BLK)
                    psk = pp1.tile([C, BLK], F32, tag="psk")
                    nc.tensor.matmul(
                        psk[:], wk_t[:], fds_t[:, sl], start=True, stop=True
                    )
                    nc.scalar.copy(kd8_t[:, sl], psk[:])

                # ---- phase 1b2: Q (pool+AQ folded; bias via kd8 row 64) ----
                rgb_r = rgb_t[:].rearrange(
                    "p (r a x c) -> p r a x c", r=HD, a=2, x=HD, c=2
                )
                for b in range(NB):
                    sl = slice(b * BLK, (b + 1) * BLK)
                    psq = pp1.tile([C, BLK], F32, tag="psq")
                    k = 0
                    for dy in range(2):
                        for dx in range(2):
                            nc.tensor.matmul(
                                psq[:],
                                wq_t[:],
                                rgb_r[:, 8 * b : 8 * b + 8, dy, :, dx],
                                start=(k == 0),
                                stop=(k == 3),
                            )
                            k += 1
                    nc.vector.tensor_copy(qd8_t[:, sl], psq[:])

                # ---- phase 1c: V'8^T tiles (4 m-tiles per psum tile) ----
                nc.gpsimd.memset(one8_t[:], 1.0)
                for gv in range(8):
                    psv = ppv.tile([128, 4 * C], F32, tag="psv")
                    for j in range(4):
                        mt = 4 * gv + j
                        nc.tensor.matmul(
                            psv[:, j * C : (j + 1) * C],
                            fds_t[:, mt * 128 : (mt + 1) * 128],
                            wv_t[:],
                            start=True,
                            stop=True,
                        )
                    csl = slice(gv * 4 * C, (gv + 1) * 4 * C)
                    nc.scalar.copy(vt8_t[:, csl], psv[:])

            if taps:
                nc.sync.dma_start(fds_o, fds_t[:])
                nc.sync.dma_start(vt_o, vt8_t[:])

            # ---- phase 2: attention + epilogue ----
            with (
                tc.tile_pool(name="et", bufs=4) as etp,
                tc.tile_pool(name="epi1", bufs=1) as epi1,
                tc.tile_pool(name="epi2", bufs=2) as epi2,
                tc.tile_pool(name="fin", bufs=2) as fin,
                tc.tile_pool(name="otp", bufs=2) as otp,
                tc.tile_pool(name="ps2", bufs=3, space="PSUM") as ps2,
                tc.tile_pool(name="avp", bufs=1, space="PSUM") as avp,
            ):
                prev = {}
                av_tiles = {}
                pending = []   # software-pipelined AV stage
                AV_DELAY = 2

                def issue_av(item):
                    av, b, g, et = item
                    etv = et[:].rearrange("m (a n) -> m a n", a=2)
                    nc.tensor.matmul(
                        av[:, 0:BLK],
                        vt8_t[:, g * 2 * C : (g * 2 + 2) * C]
                        .rearrange("m (a c) -> m a c", a=2),
                        etv,
                        start=(g == 0),
                        stop=(g == 15),
                        perf_mode=DR,
                    )
                    # denominator, broadcast over 64 partitions by a ones
                    # lhsT — no DRAM bounce needed for the reciprocal
                    nc.tensor.matmul(
                        av[:, BLK : 2 * BLK],
                        one8_t[:].rearrange("m (a c) -> m a c", a=2),
                        etv,
                        start=(g == 0),
                        stop=(g == 15),
                        perf_mode=DR,
                    )
                    if g == 15:
                        stage_norm(b)
                        if b % 2 == 1:
                            epilogue(b // 2)

                t56_tiles = {}

                def stage_norm(b):
                    """Per-block reciprocal + normalize as soon as block b's
                    AV accumulation stops (frees the av psum quickly)."""
                    p = b // 2
                    if b % 2 == 0:
                        t56 = epi1.tile([C, 1024], BF16, tag="t56")
                        t56_tiles[p] = t56
                    t56 = t56_tiles[p]
                    av = av_tiles.pop(b)
                    rbs = epi1.tile([C, BLK], F32, tag="rbs")
                    nc.vector.reciprocal_approx_fast(
                        out=rbs[:], in_=av[:, BLK : 2 * BLK]
                    )
                    h = (b % 2) * BLK
                    nc.vector.scalar_tensor_tensor(
                        t56[:, h : h + BLK], av[:, 0:BLK], SVC, rbs[:],
                        ALU.mult, ALU.mult,
                    )
                    if taps:
                        nc.sync.dma_start(
                            av_o[:, b * BLK : (b + 1) * BLK], av[:, 0:BLK]
                        )

                def attn_block(b):
                    """QK + exp for block b; AV lags AV_DELAY tiles behind."""
                    av = avp.tile([C, 2 * BLK], F32, tag="av")
                    av_tiles[b] = av
                    nsl = slice(b * BLK, (b + 1) * BLK)
                    qv = (
                        qd8_t[:, nsl]
                        .rearrange("k (o n) -> k o n", o=1)
                        .to_broadcast((C, 2, BLK))
                    )
                    for g in range(16):
                        ps = ps2.tile([128, 1024], F32, tag="ps")
                        for j in range(2):
                            mt = 2 * g + j
                            kv = (
                                kd8_t[:, mt * 128 : (mt + 1) * 128]
                                .rearrange("k (o m) -> k o m", o=1)
                                .to_broadcast((C, 2, 128))
                            )
                            nc.tensor.matmul(
                                ps[:, j * BLK : (j + 1) * BLK],
                                kv,
                                qv,
                                start=True,
                                stop=True,
                                perf_mode=DR,
                            )
                        et = etp.tile([128, 1024], FP8, tag="et")
                        nc.scalar.activation(
                            et[:, 0:ECOLS], ps[:, 0:ECOLS], AF.Exp, scale=SIG
                        )
                        nc.vector._custom_dve(
                            EXP_OP, out=et[:, ECOLS:1024], in0=ps[:, ECOLS:1024],
                            s0=EC0, s1=EC1 * SIG, imm2=EC2 * SIG * SIG,
                        )
                        pending.append((av, b, g, et))
                        while len(pending) > AV_DELAY:
                            issue_av(pending.pop(0))

                def finalize(q):
                    """LReLU + residual + output DMA for pair q's v tile,
                    processed in two half-pair chunks so DVE/Pool/DMA
                    pipeline; the last pair keeps max on DVE for tail."""
                    pv = prev["v"]
                    tail = q == NP - 1
                    for h in range(2):
                        hsl = slice(h * 2048, (h + 1) * 2048)
                        l02 = fin.tile([C, 2048], BF16, tag="l02")
                        nc.vector.tensor_scalar(
                            l02[:], pv[:, hsl], NEG_SLOPE, None, ALU.mult
                        )
                        y4 = fin.tile([C, 2048], BF16, tag="y4")
                        nc.vector.tensor_tensor(
                            y4[:], pv[:, hsl], l02[:], ALU.max
                        )
                        osl = slice(q * 4096 + h * 2048, q * 4096 + (h + 1) * 2048)
                        ot = otp.tile([C, 2048], BF16, tag="ot")
                        otv = ot[:].rearrange(
                            "c (r x a) -> c r x a", r=16, x=HD, a=2
                        )
                        y4v = y4[:].rearrange(
                            "c (r a x) -> c r a x", r=16, a=2, x=HD
                        )
                        rgv = rgb_t[:, osl].rearrange(
                            "c (r x a) -> c r x a", r=16, x=HD, a=2
                        )
                        nc.gpsimd.tensor_tensor(
                            otv[:, :, :, 0], y4v[:, :, 0, :], rgv[:, :, :, 0],
                            ALU.add,
                        )
                        nc.gpsimd.tensor_tensor(
                            otv[:, :, :, 1], y4v[:, :, 1, :], rgv[:, :, :, 1],
                            ALU.add,
                        )
                        nc.sync.dma_start(out_d[:, osl], ot[:])
                    if taps:
                        nc.sync.dma_start(
                            v_o[:, q * 4096 : (q + 1) * 4096], pv[:]
                        )

                def epilogue(p):
                    b0, b1 = 2 * p, 2 * p + 1
                    t56 = t56_tiles.pop(p)
                    if taps:
                        nc.sync.dma_start(t56_o[:, b0 * BLK : (b1 + 1) * BLK],
                                          t56[:])
                    # p18 = t56/3 + 0.75*b'  (carries the conv/BN bias)
                    p18 = epi1.tile([C, 1024], BF16, tag="p18")
                    nc.vector.tensor_scalar(
                        p18[:], t56[:], 1.0 / 3.0, b75_t[:], ALU.mult, ALU.add
                    )

                    # x-upsample: xup75 = 0.75*(xup + b'), layout [r16, par2, x64]
                    xup = epi2.tile([C, 2048], BF16, tag="xup")
                    xv = xup[:].rearrange("c (r a x) -> c r a x", r=16, a=2, x=HD)
                    t56v = t56[:].rearrange("c (r x) -> c r x", r=16, x=HD)
                    p18v = p18[:].rearrange("c (r x) -> c r x", r=16, x=HD)
                    # even out col 2i: p18[i-1] + t56[i] (i>=1); i=0 clamps
                    nc.gpsimd.tensor_tensor(
                        xv[:, :, 0, 1:64], p18v[:, :, 0:63], t56v[:, :, 1:64],
                        ALU.add,
                    )
                    nc.gpsimd.tensor_tensor(
                        xv[:, :, 0, 0:1], p18v[:, :, 0:1], t56v[:, :, 0:1],
                        ALU.add,
                    )
                    # odd out col 2i+1: t56[i] + p18[i+1] (i<=62); i=63 clamps
                    nc.gpsimd.tensor_tensor(
                        xv[:, :, 1, 0:63], t56v[:, :, 0:63], p18v[:, :, 1:64],
                        ALU.add,
                    )
                    nc.gpsimd.tensor_tensor(
                        xv[:, :, 1, 63:64], t56v[:, :, 63:64], p18v[:, :, 63:64],
                        ALU.add,
                    )
                    if taps:
                        nc.sync.dma_start(
                            xup_o[:, b0 * 1024 : (b1 + 1) * 1024], xup[:]
                        )
                    # x18 = xup75/3
                    x18 = epi2.tile([C, 2048], BF16, tag="x18")
                    nc.vector.tensor_scalar(
                        x18[:], xup[:], 1.0 / 3.0, None, ALU.mult
                    )

                    # y-upsample rows: v[r'] layout [r'32, 128]
                    v = epi2.tile([C, 4096], BF16, tag="v")
                    vv = v[:].rearrange("c (r w) -> c r w", r=32, w=H)
                    xr = xup[:].rearrange("c (r w) -> c r w", r=16, w=H)
                    x18r = x18[:].rearrange("c (r w) -> c r w", r=16, w=H)
                    # even rows 2j = x18[j-1] + xup75[j], j=1..15
                    nc.gpsimd.tensor_tensor(
                        vv[:, 2:32:2, :], x18r[:, 0:15, :], xr[:, 1:16, :],
                        ALU.add,
                    )
                    # even row 0: boundary with previous pair (or clamp)
                    if p == 0:
                        nc.vector.tensor_tensor(
                            vv[:, 0:1, :], x18r[:, 0:1, :], xr[:, 0:1, :],
                            ALU.add,
                        )
                    else:
                        pxr18 = prev["x18"][:].rearrange(
                            "c (r w) -> c r w", r=16, w=H
                        )
                        nc.vector.tensor_tensor(
                            vv[:, 0:1, :], pxr18[:, 15:16, :], xr[:, 0:1, :],
                            ALU.add,
                        )
                        # previous pair's last row: xup75_prev[15] + x18[0]
                        pvv = prev["v"][:].rearrange("c (r w) -> c r w", r=32, w=H)
                        pxr = prev["xup"][:].rearrange(
                            "c (r w) -> c r w", r=16, w=H
                        )
                        nc.vector.tensor_tensor(
                            pvv[:, 31:32, :], pxr[:, 15:16, :], x18r[:, 0:1, :],
                            ALU.add,
                        )
                        finalize(p - 1)
                    # odd rows 2j+1 = xup75[j] + x18[j+1], j=0..14
                    nc.gpsimd.tensor_tensor(
                        vv[:, 1:31:2, :], xr[:, 0:15, :], x18r[:, 1:16, :],
                        ALU.add,
                    )
                    if p == NP - 1:
                        # last image row clamps: xup75[15] + x18[15]
                        nc.vector.tensor_tensor(
                            vv[:, 31:32, :], xr[:, 15:16, :], x18r[:, 15:16, :],
                            ALU.add,
                        )
                    prev.update(v=v, xup=xup, x18=x18)

                for b in range(NB):
                    attn_block(b)
                while pending:
                    issue_av(pending.pop(0))
                finalize(NP - 1)
                if taps:
                    nc.sync.dma_start(qd_o, qd8_t[:])
                    nc.sync.dma_start(kd_o, kd8_t[:])

    nc.compile()
    return nc, None


def _prep_weights(w_q, b_q, w_k, b_k, w_v, b_v, w_o, b_o, bn_gamma, bn_beta,
                  bn_mean, bn_var):
    bf = ml_dtypes.bfloat16
    f8 = ml_dtypes.float8_e4m3
    inv = bn_gamma / np.sqrt(bn_var + BN_EPS)
    wo_p = w_o * inv[:, None]                     # BN-folded conv weight
    bprime = inv * (b_o - bn_mean) + bn_beta      # BN-folded conv bias

    # Q: pool(0.25) and AQ folded; bias handled via the wk2 extra column
    wq_l = (w_q.T * (0.25 * AQ)).astype(bf)
    # K: AK folded (q-bias dropped: costs ~5e-5 rel err, saves the DR
    # contraction row)
    wk_l = (np.vstack([w_k.T, b_k[None, :]]) * AK).astype(bf)
    # V': wv2 = wv_l @ M folds conv into V and keeps the ones channel
    wv_l = np.zeros((C + 1, C + 1), np.float32)
    wv_l[0:C, 0:C] = w_v.T
    wv_l[C, 0:C] = b_v
    wv_l[C, C] = 1.0
    M = np.zeros((C + 1, C), np.float32)
    M[0:C, 0:C] = SV * wo_p.T
    wv2 = (wv_l @ M).astype(bf)
    eye = 0.25 * np.eye(C, dtype=np.float32)
    wi_l = np.vstack([eye, eye]).astype(f8)
    b75 = (0.75 * bprime)[:, None].astype(np.float32)
    return dict(wq_l=wq_l, wk_l=wk_l, wv2=wv2, wi_l=wi_l, b75=b75,
                onesb=np.ones((1, N), bf))


_CACHED = {}


def kernel(**inputs):
    bf = ml_dtypes.bfloat16
    f8 = ml_dtypes.float8_e4m3
    rgb = np.asarray(inputs["rgb"], np.float32)
    freq = np.asarray(inputs["freq"], np.float32)
    wts = _prep_weights(
        np.asarray(inputs["w_q"], np.float32), np.asarray(inputs["b_q"], np.float32),
        np.asarray(inputs["w_k"], np.float32), np.asarray(inputs["b_k"], np.float32),
        np.asarray(inputs["w_v"], np.float32), np.asarray(inputs["b_v"], np.float32),
        np.asarray(inputs["w_o"], np.float32), np.asarray(inputs["b_o"], np.float32),
        np.asarray(inputs["bn_gamma"], np.float32),
        np.asarray(inputs["bn_beta"], np.float32),
        np.asarray(inputs["bn_mean"], np.float32),
        np.asarray(inputs["bn_var"], np.float32),
    )
    if "nc" not in _CACHED:
        _CACHED["nc"], _ = build_program()
    nc = _CACHED["nc"]
    in_maps = []
    for i in range(B):
        m = dict(wts)
        m["rgb"] = np.ascontiguousarray(rgb[i].reshape(C, HW)).astype(bf)
        m["freq"] = np.ascontiguousarray(freq[i].reshape(C, HW)).astype(f8)
        in_maps.append(m)
    res = run_bass_kernel_spmd(nc, in_maps, list(range(B)))
    out = np.stack([res.results[i]["out"] for i in range(B)])
    return out.reshape(B, C, H, H).astype(np.float32)


if __name__ == "__main__":
    nc, _ = build_program()
    print("program built OK")


# revision 37
# speedup vs baseline: 1.0077x; 1.0077x over previous
"""Trainium2 Bass kernel for nn_CMA_Block (cross-modal attention block).

Per-sample pipeline (data-parallel over B=8 across 8 NeuronCores):
  rgb(bf16),freq(fp8) -> avgpool2 -> QKV 1x1-conv projections (pool folded
  into accumulating matmuls; q/k quantized to fp8 with power-of-2 scales;
  q-bias folded into a 65th k-row via wk2 = [wk | wk@qb]) ->
  S^T = K^T Q via fp8 DoubleRow matmuls (stride-0 broadcast pair dims;
  exp-scale folded into the activation scale) -> exp on ACT/DVE -> fp8
  DoubleRow AV with V' = Wo'V pre-folded (conv1x1 eliminated; ones channel
  gives the softmax denominator) -> normalize + 2x bilinear upsample as a
  prescale/strided-add chain on DVE fast modes -> LeakyReLU -> residual add
  on GPSIMD -> bf16 output DMA (host converts to f32).
"""

import sys

sys.path.insert(0, "/opt/trn_rl_repo")

import numpy as np
import ml_dtypes

import concourse.bass as bass
import concourse.bacc as bacc
import concourse.mybir as mybir
import concourse.tile as tile
from concourse.bass_utils import run_bass_kernel_spmd
import concourse.dve_ops as dve_ops
from concourse.dve_spec import (
    Spec, Src0, C0, C1, C2, sq, lower, _has_src1 as has_src1,
)
from concourse.dve_uop import DveOpSpec

# exp(x) ~= ((EC2*x + EC1)*x + EC0)^16, max rel err 5.5e-4 on [-1.5, 1.5]
EC0, EC1, EC2 = 1.0000024, 0.06256861, 0.00195205


def _register_exp_op():
    """Register a one-pass DVE polynomial exp (quadratic seed + 4 squarings)."""
    name = "EXP_POLY16_ANT"
    for op in dve_ops.OPS:
        if op.name == name:
            return op
    body = sq(sq(sq(sq((Src0 * C2 + C1) * Src0 + C0))))
    spec = Spec(
        body=body,
        reference=lambda in0, in1, s0, s1, imm2: (
            (((in0 * imm2 + s1) * in0 + s0)) ** 16
        ).astype(np.float32),
    )
    row = dve_ops._CUSTOM_DVE_ROW_BASE + len(dve_ops.OPS)
    dve_ops._SUB_OPCODE_FOR_NAME[name] = row
    shas = {}
    for ver in ("v3", "v4"):
        sp = DveOpSpec(
            name=name, opcode=row, uops=lower(spec, ver=ver),
            rd1_en=has_src1(spec),
        )
        shas[ver] = sp.sha(ver)
    op = dve_ops.DveOp(name, spec, subdim=False, uops_sha=shas)
    dve_ops.OPS.append(op)
    dve_ops.CUSTOM_DVE_SPECS[name] = spec
    return op


EXP_OP = _register_exp_op()

F32 = mybir.dt.float32
F32R = mybir.dt.float32r
BF16 = mybir.dt.bfloat16
FP8 = mybir.dt.float8e4
AF = mybir.ActivationFunctionType
ALU = mybir.AluOpType
DR = mybir.MatmulPerfMode.DoubleRow

# Problem shape constants (hardcoded per contract).
B = 8          # batch == n_cores
C = 64         # channels (Cin == Hid == Cout == 64)
H = 128        # full-res H == W
HW = H * H     # 16384
HD = 64        # pooled H == W
N = HD * HD    # 4096 tokens
NB = 8         # n-blocks of 512 tokens
BLK = N // NB  # 512
MT = 32        # m-tiles of 128 tokens
NP = 4         # block pairs (1024 tokens each)
NEG_SLOPE = 0.2
BN_EPS = 1e-5

# fp8 scale plan: q8 = AQ*q_raw, k8 = AK*k_raw; stride-0 DoubleRow doubles
# the product; exp() folds sigma = attn_scale / (AQ*AK*2) back in.
AQ = 4.0
AK = 4.0
SIG = (C ** -0.5) / (AQ * AK * 2.0)   # 2^-8
SV = 4.0                              # V'8 = SV * Wo' V
SVC = 0.5625 / SV                     # stt scalar: 0.5625/SV

# exp engine split: each [128,1024] tile is split column-wise, ACT takes
# the first ECOLS columns, DVE (custom poly op) the rest — both engines
# run in lockstep on every tile.
ECOLS = 656


def build_program(debug=False, taps=False):
    """Build the per-core (SPMD) bass program. Returns (nc, io_names)."""
    nc = bacc.Bacc(
        "TRN2",
        target_bir_lowering=False,
        debug=debug,
        enable_asserts=False,
        num_devices=B,
    )

    # DRAM I/O (per-core slices of the batch; weights replicated).
    rgb_d = nc.dram_tensor("rgb", [C, HW], BF16, kind="ExternalInput").ap()
    freq_d = nc.dram_tensor("freq", [C, HW], FP8, kind="ExternalInput").ap()
    wq_d = nc.dram_tensor("wq_l", [C, C], BF16, kind="ExternalInput").ap()
    wk_d = nc.dram_tensor("wk_l", [C + 1, C], BF16, kind="ExternalInput").ap()
    wv_d = nc.dram_tensor("wv2", [C + 1, C], BF16, kind="ExternalInput").ap()
    wi_d = nc.dram_tensor("wi_l", [2 * C, C], FP8, kind="ExternalInput").ap()
    b75_d = nc.dram_tensor("b75", [C, 1], F32, kind="ExternalInput").ap()
    onesb_d = nc.dram_tensor("onesb", [1, N], BF16, kind="ExternalInput").ap()
    out_d = nc.dram_tensor("out", [C, HW], BF16, kind="ExternalOutput").ap()
    recd = nc.dram_tensor("rec_scratch", [NB, BLK], F32).ap()
    if taps:
        fds_o = nc.dram_tensor("fds_o", [C + 1, N], BF16, kind="ExternalOutput").ap()
        qd_o = nc.dram_tensor("qd_o", [C, N], FP8, kind="ExternalOutput").ap()
        kd_o = nc.dram_tensor("kd_o", [C, N], FP8, kind="ExternalOutput").ap()
        vt_o = nc.dram_tensor("vt_o", [128, MT * C], FP8,
                              kind="ExternalOutput").ap()
        av_o = nc.dram_tensor("av_o", [C + 1, N], F32, kind="ExternalOutput").ap()
        t56_o = nc.dram_tensor("t56_o", [C, N], BF16, kind="ExternalOutput").ap()
        xup_o = nc.dram_tensor("xup_o", [C, 2 * N], BF16, kind="ExternalOutput").ap()
        v_o = nc.dram_tensor("v_o", [C, HW], BF16, kind="ExternalOutput").ap()

    with tile.TileContext(nc) as tc:
        with (
            tc.tile_pool(name="const", bufs=1) as cpool,
            tc.tile_pool(name="persist", bufs=1) as perm,
        ):
            # ---- constants ----
            wq_t = cpool.tile([C, C], BF16, tag="wq")
            wk_t = cpool.tile([C + 1, C], BF16, tag="wk")
            wv_t = cpool.tile([C + 1, C], BF16, tag="wv")
            wi_t = cpool.tile([2 * C, C], FP8, tag="wi")
            b75_t = cpool.tile([C, 1], F32, tag="b75")
            nc.sync.dma_start(wq_t[:], wq_d)
            nc.sync.dma_start(wk_t[:], wk_d)
            nc.sync.dma_start(wv_t[:], wv_d)
            nc.sync.dma_start(wi_t[:], wi_d)
            nc.sync.dma_start(b75_t[:], b75_d)

            # ---- persistent SBUF tensors ----
            rgb_t = perm.tile([C, HW], BF16, tag="rgb")      # Q rhs + residual
            fds_t = perm.tile([C + 1, N], BF16, tag="fds")   # pooled freq +ones
            qd8_t = perm.tile([C, N], FP8, tag="qd8")        # q8 fp8
            kd8_t = perm.tile([C, N], FP8, tag="kd8")        # k8 fp8
            vt8_t = perm.tile([128, MT * C], FP8, tag="vt8")  # V'8^T tiles
            one8_t = perm.tile([128, 128], FP8, tag="one8")  # DR den-dup lhsT

            for p in range(NP):
                sl = slice(p * 4096, (p + 1) * 4096)
                nc.sync.dma_start(rgb_t[:, sl], rgb_d[:, sl])
            nc.gpsimd.dma_start(fds_t[C : C + 1, :], onesb_d)

            with (
                tc.tile_pool(name="p1sb", bufs=1) as p1sb,
                tc.tile_pool(name="pp1", bufs=2, space="PSUM") as pp1,
                tc.tile_pool(name="ppv", bufs=2, space="PSUM") as ppv,
            ):
                freq_t = p1sb.tile([C, HW], FP8, tag="freq")
                for p in range(NP):
                    sl = slice(p * 4096, (p + 1) * 4096)
                    nc.scalar.dma_start(freq_t[:, sl], freq_d[:, sl])

                # ---- phase 1a: pool freq via 4 accumulating fp8 matmuls ----
                freq_r = freq_t[:].rearrange(
                    "p (r a x c) -> p r a x c", r=HD, a=2, x=HD, c=2
                )
                for b in range(NB):
                    sl = slice(b * BLK, (b + 1) * BLK)
                    psf = pp1.tile([C, BLK], F32, tag="psf")
                    k = 0
                    for dy in range(2):
                        for dx in range(2):
                            nc.tensor.matmul(
                                psf[:],
                                wi_t[0:C, :],
                                freq_r[:, 8 * b : 8 * b + 8, dy, :, dx],
                                start=(k == 0),
                                stop=(k == 3),
                            )
                            k += 1
                    nc.vector.tensor_copy(fds_t[0:C, sl], psf[:])

                # ---- phase 1b: K (wk2 includes q-bias row as output 64) ----
                for b in range(NB):
                    sl = slice(b * BLK, (b + 1) * BLK)
                    psk = pp1.tile([C, BLK], F32, tag="psk")
                    nc.tensor.matmul(
                        psk[:], wk_t[:], fds_t[:, sl], start=True, stop=True
                    )
                    nc.scalar.copy(kd8_t[:, sl], psk[:])

                # ---- phase 1b2: Q (pool+AQ folded; bias via kd8 row 64) ----
                rgb_r = rgb_t[:].rearrange(
                    "p (r a x c) -> p r a x c", r=HD, a=2, x=HD, c=2
                )
                for b in range(NB):
                    sl = slice(b * BLK, (b + 1) * BLK)
                    psq = pp1.tile([C, BLK], F32, tag="psq")
                    k = 0
                    for dy in range(2):
                        for dx in range(2):
                            nc.tensor.matmul(
                                psq[:],
                                wq_t[:],
                                rgb_r[:, 8 * b : 8 * b + 8, dy, :, dx],
                                start=(k == 0),
                                stop=(k == 3),
                            )
                            k += 1
                    nc.vector.tensor_copy(qd8_t[:, sl], psq[:])

                # ---- phase 1c: V'8^T tiles (4 m-tiles per psum tile) ----
                nc.gpsimd.memset(one8_t[:], 1.0)
                for gv in range(8):
                    psv = ppv.tile([128, 4 * C], F32, tag="psv")
                    for j in range(4):
                        mt = 4 * gv + j
                        nc.tensor.matmul(
                            psv[:, j * C : (j + 1) * C],
                            fds_t[:, mt * 128 : (mt + 1) * 128],
                            wv_t[:],
                            start=True,
                            stop=True,
                        )
                    csl = slice(gv * 4 * C, (gv + 1) * 4 * C)
                    nc.scalar.copy(vt8_t[:, csl], psv[:])

            if taps:
                nc.sync.dma_start(fds_o, fds_t[:])
                nc.sync.dma_start(vt_o, vt8_t[:])

            # ---- phase 2: attention + epilogue ----
            with (
                tc.tile_pool(name="et", bufs=5) as etp,
                tc.tile_pool(name="epi1", bufs=1) as epi1,
                tc.tile_pool(name="epi2", bufs=2) as epi2,
                tc.tile_pool(name="fin", bufs=2) as fin,
                tc.tile_pool(name="otp", bufs=2) as otp,
                tc.tile_pool(name="ps2", bufs=3, space="PSUM") as ps2,
                tc.tile_pool(name="avp", bufs=1, space="PSUM") as avp,
            ):
                prev = {}
                av_tiles = {}
                pending = []   # software-pipelined AV stage
                AV_DELAY = 3

                def issue_av(item):
                    av, b, g, et = item
                    etv = et[:].rearrange("m (a n) -> m a n", a=2)
                    nc.tensor.matmul(
                        av[:, 0:BLK],
                        vt8_t[:, g * 2 * C : (g * 2 + 2) * C]
                        .rearrange("m (a c) -> m a c", a=2),
                        etv,
                        start=(g == 0),
                        stop=(g == 15),
                        perf_mode=DR,
                    )
                    # denominator, broadcast over 64 partitions by a ones
                    # lhsT — no DRAM bounce needed for the reciprocal
                    nc.tensor.matmul(
                        av[:, BLK : 2 * BLK],
                        one8_t[:].rearrange("m (a c) -> m a c", a=2),
                        etv,
                        start=(g == 0),
                        stop=(g == 15),
                        perf_mode=DR,
                    )
                    if g == 15:
                        stage_norm(b)
                        if b % 2 == 1:
                            epilogue(b // 2)

                t56_tiles = {}

                def stage_norm(b):
                    """Per-block reciprocal + normalize as soon as block b's
                    AV accumulation stops (frees the av psum quickly)."""
                    p = b // 2
                    if b % 2 == 0:
                        t56 = epi1.tile([C, 1024], BF16, tag="t56")
                        t56_tiles[p] = t56
                    t56 = t56_tiles[p]
                    av = av_tiles.pop(b)
                    rbs = epi1.tile([C, BLK], F32, tag="rbs")
                    nc.vector.reciprocal_approx_fast(
                        out=rbs[:], in_=av[:, BLK : 2 * BLK]
                    )
                    h = (b % 2) * BLK
                    nc.vector.scalar_tensor_tensor(
                        t56[:, h : h + BLK], av[:, 0:BLK], SVC, rbs[:],
                        ALU.mult, ALU.mult,
                    )
                    if taps:
                        nc.sync.dma_start(
                            av_o[:, b * BLK : (b + 1) * BLK], av[:, 0:BLK]
                        )

                def attn_block(b):
                    """QK + exp for block b; AV lags AV_DELAY tiles behind."""
                    av = avp.tile([C, 2 * BLK], F32, tag="av")
                    av_tiles[b] = av
                    nsl = slice(b * BLK, (b + 1) * BLK)
                    qv = (
                        qd8_t[:, nsl]
                        .rearrange("k (o n) -> k o n", o=1)
                        .to_broadcast((C, 2, BLK))
                    )
                    for g in range(16):
                        ps = ps2.tile([128, 1024], F32, tag="ps")
                        for j in range(2):
                            mt = 2 * g + j
                            kv = (
                                kd8_t[:, mt * 128 : (mt + 1) * 128]
                                .rearrange("k (o m) -> k o m", o=1)
                                .to_broadcast((C, 2, 128))
                            )
                            nc.tensor.matmul(
                                ps[:, j * BLK : (j + 1) * BLK],
                                kv,
                                qv,
                                start=True,
                                stop=True,
                                perf_mode=DR,
                            )
                        et = etp.tile([128, 1024], FP8, tag="et")
                        nc.scalar.activation(
                            et[:, 0:ECOLS], ps[:, 0:ECOLS], AF.Exp, scale=SIG
                        )
                        nc.vector._custom_dve(
                            EXP_OP, out=et[:, ECOLS:1024], in0=ps[:, ECOLS:1024],
                            s0=EC0, s1=EC1 * SIG, imm2=EC2 * SIG * SIG,
                        )
                        pending.append((av, b, g, et))
                        while len(pending) > AV_DELAY:
                            issue_av(pending.pop(0))

                def finalize(q):
                    """LReLU + residual + output DMA for pair q's v tile,
                    processed in two half-pair chunks so DVE/Pool/DMA
                    pipeline; the last pair keeps max on DVE for tail."""
                    pv = prev["v"]
                    tail = q == NP - 1
                    for h in range(2):
                        hsl = slice(h * 2048, (h + 1) * 2048)
                        l02 = fin.tile([C, 2048], BF16, tag="l02")
                        nc.vector.tensor_scalar(
                            l02[:], pv[:, hsl], NEG_SLOPE, None, ALU.mult
                        )
                        y4 = fin.tile([C, 2048], BF16, tag="y4")
                        nc.vector.tensor_tensor(
                            y4[:], pv[:, hsl], l02[:], ALU.max
                        )
                        osl = slice(q * 4096 + h * 2048, q * 4096 + (h + 1) * 2048)
                        ot = otp.tile([C, 2048], BF16, tag="ot")
                        otv = ot[:].rearrange(
                            "c (r x a) -> c r x a", r=16, x=HD, a=2
                        )
                        y4v = y4[:].rearrange(
                            "c (r a x) -> c r a x", r=16, a=2, x=HD
                        )
                        rgv = rgb_t[:, osl].rearrange(
                            "c (r x a) -> c r x a", r=16, x=HD, a=2
                        )
                        nc.gpsimd.tensor_tensor(
                            otv[:, :, :, 0], y4v[:, :, 0, :], rgv[:, :, :, 0],
                            ALU.add,
                        )
                        nc.gpsimd.tensor_tensor(
                            otv[:, :, :, 1], y4v[:, :, 1, :], rgv[:, :, :, 1],
                            ALU.add,
                        )
                        nc.sync.dma_start(out_d[:, osl], ot[:])
                    if taps:
                        nc.sync.dma_start(
                            v_o[:, q * 4096 : (q + 1) * 4096], pv[:]
                        )

                def epilogue(p):
                    b0, b1 = 2 * p, 2 * p + 1
                    t56 = t56_tiles.pop(p)
                    if taps:
                        nc.sync.dma_start(t56_o[:, b0 * BLK : (b1 + 1) * BLK],
                                          t56[:])
                    # p18 = t56/3 + 0.75*b'  (carries the conv/BN bias)
                    p18 = epi1.tile([C, 1024], BF16, tag="p18")
                    nc.vector.tensor_scalar(
                        p18[:], t56[:], 1.0 / 3.0, b75_t[:], ALU.mult, ALU.add
                    )

                    # x-upsample: xup75 = 0.75*(xup + b'), layout [r16, par2, x64]
                    xup = epi2.tile([C, 2048], BF16, tag="xup")
                    xv = xup[:].rearrange("c (r a x) -> c r a x", r=16, a=2, x=HD)
                    t56v = t56[:].rearrange("c (r x) -> c r x", r=16, x=HD)
                    p18v = p18[:].rearrange("c (r x) -> c r x", r=16, x=HD)
                    # even out col 2i: p18[i-1] + t56[i] (i>=1); i=0 clamps
                    nc.gpsimd.tensor_tensor(
                        xv[:, :, 0, 1:64], p18v[:, :, 0:63], t56v[:, :, 1:64],
                        ALU.add,
                    )
                    nc.gpsimd.tensor_tensor(
                        xv[:, :, 0, 0:1], p18v[:, :, 0:1], t56v[:, :, 0:1],
                        ALU.add,
                    )
                    # odd out col 2i+1: t56[i] + p18[i+1] (i<=62); i=63 clamps
                    nc.gpsimd.tensor_tensor(
                        xv[:, :, 1, 0:63], t56v[:, :, 0:63], p18v[:, :, 1:64],
                        ALU.add,
                    )
                    nc.gpsimd.tensor_tensor(
                        xv[:, :, 1, 63:64], t56v[:, :, 63:64], p18v[:, :, 63:64],
                        ALU.add,
                    )
                    if taps:
                        nc.sync.dma_start(
                            xup_o[:, b0 * 1024 : (b1 + 1) * 1024], xup[:]
                        )
                    # x18 = xup75/3
                    x18 = epi2.tile([C, 2048], BF16, tag="x18")
                    nc.vector.tensor_scalar(
                        x18[:], xup[:], 1.0 / 3.0, None, ALU.mult
                    )

                    # y-upsample rows: v[r'] layout [r'32, 128]
                    v = epi2.tile([C, 4096], BF16, tag="v")
                    vv = v[:].rearrange("c (r w) -> c r w", r=32, w=H)
                    xr = xup[:].rearrange("c (r w) -> c r w", r=16, w=H)
                    x18r = x18[:].rearrange("c (r w) -> c r w", r=16, w=H)
                    # even rows 2j = x18[j-1] + xup75[j], j=1..15
                    nc.gpsimd.tensor_tensor(
                        vv[:, 2:32:2, :], x18r[:, 0:15, :], xr[:, 1:16, :],
                        ALU.add,
                    )
                    # even row 0: boundary with previous pair (or clamp)
                    if p == 0:
                        nc.vector.tensor_tensor(
                            vv[:, 0:1, :], x18r[:, 0:1, :], xr[:, 0:1, :],
                            ALU.add,
                        )
                    else:
                        pxr18 = prev["x18"][:].rearrange(
                            "c (r w) -> c r w", r=16, w=H
                        )
                        nc.vector.tensor_tensor(
                            vv[:, 0:1, :], pxr18[:, 15:16, :], xr[:, 0:1, :],
                            ALU.add,
                        )
                        # previous pair's last row: xup75_prev[15] + x18[0]
                        pvv = prev["v"][:].rearrange("c (r w) -> c r w", r=32, w=H)
                        pxr = prev["xup"][:].rearrange(
                            "c (r w) -> c r w", r=16, w=H
                        )
                        nc.vector.tensor_tensor(
                            pvv[:, 31:32, :], pxr[:, 15:16, :], x18r[:, 0:1, :],
                            ALU.add,
                        )
                        finalize(p - 1)
                    # odd rows 2j+1 = xup75[j] + x18[j+1], j=0..14
                    nc.gpsimd.tensor_tensor(
                        vv[:, 1:31:2, :], xr[:, 0:15, :], x18r[:, 1:16, :],
                        ALU.add,
                    )
                    if p == NP - 1:
                        # last image row clamps: xup75[15] + x18[15]
                        nc.vector.tensor_tensor(
                            vv[:, 31:32, :], xr[:, 15:16, :], x18r[:, 15:16, :],
                            ALU.add,
                        )
                    prev.update(v=v, xup=xup, x18=x18)

                for b in range(NB):
                    attn_block(b)
                while pending:
                    issue_av(pending.pop(0))
                finalize(NP - 1)
                if taps:
                    nc.sync.dma_start(qd_o, qd8_t[:])
                    nc.sync.dma_start(kd_o, kd8_t[:])

    nc.compile()
    return nc, None


def _prep_weights(w_q, b_q, w_k, b_k, w_v, b_v, w_o, b_o, bn_gamma, bn_beta,
                  bn_mean, bn_var):
    bf = ml_dtypes.bfloat16
    f8 = ml_dtypes.float8_e4m3
    inv = bn_gamma / np.sqrt(bn_var + BN_EPS)
    wo_p = w_o * inv[:, None]                     # BN-folded conv weight
    bprime = inv * (b_o - bn_mean) + bn_beta      # BN-folded conv bias

    # Q: pool(0.25) and AQ folded; bias handled via the wk2 extra column
    wq_l = (w_q.T * (0.25 * AQ)).astype(bf)
    # K: AK folded (q-bias dropped: costs ~5e-5 rel err, saves the DR
    # contraction row)
    wk_l = (np.vstack([w_k.T, b_k[None, :]]) * AK).astype(bf)
    # V': wv2 = wv_l @ M folds conv into V and keeps the ones channel
    wv_l = np.zeros((C + 1, C + 1), np.float32)
    wv_l[0:C, 0:C] = w_v.T
    wv_l[C, 0:C] = b_v
    wv_l[C, C] = 1.0
    M = np.zeros((C + 1, C), np.float32)
    M[0:C, 0:C] = SV * wo_p.T
    wv2 = (wv_l @ M).astype(bf)
    eye = 0.25 * np.eye(C, dtype=np.float32)
    wi_l = np.vstack([eye, eye]).astype(f8)
    b75 = (0.75 * bprime)[:, None].astype(np.float32)
    return dict(wq_l=wq_l, wk_l=wk_l, wv2=wv2, wi_l=wi_l, b75=b75,
                onesb=np.ones((1, N), bf))


_CACHED = {}


def kernel(**inputs):
    bf = ml_dtypes.bfloat16
    f8 = ml_dtypes.float8_e4m3
    rgb = np.asarray(inputs["rgb"], np.float32)
    freq = np.asarray(inputs["freq"], np.float32)
    wts = _prep_weights(
        np.asarray(inputs["w_q"], np.float32), np.asarray(inputs["b_q"], np.float32),
        np.asarray(inputs["w_k"], np.float32), np.asarray(inputs["b_k"], np.float32),
        np.asarray(inputs["w_v"], np.float32), np.asarray(inputs["b_v"], np.float32),
        np.asarray(inputs["w_o"], np.float32), np.asarray(inputs["b_o"], np.float32),
        np.asarray(inputs["bn_gamma"], np.float32),
        np.asarray(inputs["bn_beta"], np.float32),
        np.asarray(inputs["bn_mean"], np.float32),
        np.asarray(inputs["bn_var"], np.float32),
    )
    if "nc" not in _CACHED:
        _CACHED["nc"], _ = build_program()
    nc = _CACHED["nc"]
    in_maps = []
    for i in range(B):
        m = dict(wts)
        m["rgb"] = np.ascontiguousarray(rgb[i].reshape(C, HW)).astype(bf)
        m["freq"] = np.ascontiguousarray(freq[i].reshape(C, HW)).astype(f8)
        in_maps.append(m)
    res = run_bass_kernel_spmd(nc, in_maps, list(range(B)))
    out = np.stack([res.results[i]["out"] for i in range(B)])
    return out.reshape(B, C, H, H).astype(np.float32)


if __name__ == "__main__":
    nc, _ = build_program()
    print("program built OK")


# revision 38
# speedup vs baseline: 1.0112x; 1.0034x over previous
"""Trainium2 Bass kernel for nn_CMA_Block (cross-modal attention block).

Per-sample pipeline (data-parallel over B=8 across 8 NeuronCores):
  rgb(bf16),freq(fp8) -> avgpool2 -> QKV 1x1-conv projections (pool folded
  into accumulating matmuls; q/k quantized to fp8 with power-of-2 scales;
  q-bias folded into a 65th k-row via wk2 = [wk | wk@qb]) ->
  S^T = K^T Q via fp8 DoubleRow matmuls (stride-0 broadcast pair dims;
  exp-scale folded into the activation scale) -> exp on ACT/DVE -> fp8
  DoubleRow AV with V' = Wo'V pre-folded (conv1x1 eliminated; ones channel
  gives the softmax denominator) -> normalize + 2x bilinear upsample as a
  prescale/strided-add chain on DVE fast modes -> LeakyReLU -> residual add
  on GPSIMD -> bf16 output DMA (host converts to f32).
"""

import sys

sys.path.insert(0, "/opt/trn_rl_repo")

import numpy as np
import ml_dtypes

import concourse.bass as bass
import concourse.bacc as bacc
import concourse.mybir as mybir
import concourse.tile as tile
from concourse.bass_utils import run_bass_kernel_spmd
import concourse.dve_ops as dve_ops
from concourse.dve_spec import (
    Spec, Src0, C0, C1, C2, sq, lower, _has_src1 as has_src1,
)
from concourse.dve_uop import DveOpSpec

# exp(x) ~= ((EC2*x + EC1)*x + EC0)^16, max rel err 5.5e-4 on [-1.5, 1.5]
EC0, EC1, EC2 = 1.0000024, 0.06256861, 0.00195205


def _register_exp_op():
    """Register a one-pass DVE polynomial exp (quadratic seed + 4 squarings)."""
    name = "EXP_POLY16_ANT"
    for op in dve_ops.OPS:
        if op.name == name:
            return op
    body = sq(sq(sq(sq((Src0 * C2 + C1) * Src0 + C0))))
    spec = Spec(
        body=body,
        reference=lambda in0, in1, s0, s1, imm2: (
            (((in0 * imm2 + s1) * in0 + s0)) ** 16
        ).astype(np.float32),
    )
    row = dve_ops._CUSTOM_DVE_ROW_BASE + len(dve_ops.OPS)
    dve_ops._SUB_OPCODE_FOR_NAME[name] = row
    shas = {}
    for ver in ("v3", "v4"):
        sp = DveOpSpec(
            name=name, opcode=row, uops=lower(spec, ver=ver),
            rd1_en=has_src1(spec),
        )
        shas[ver] = sp.sha(ver)
    op = dve_ops.DveOp(name, spec, subdim=False, uops_sha=shas)
    dve_ops.OPS.append(op)
    dve_ops.CUSTOM_DVE_SPECS[name] = spec
    return op


EXP_OP = _register_exp_op()

F32 = mybir.dt.float32
F32R = mybir.dt.float32r
BF16 = mybir.dt.bfloat16
FP8 = mybir.dt.float8e4
AF = mybir.ActivationFunctionType
ALU = mybir.AluOpType
DR = mybir.MatmulPerfMode.DoubleRow

# Problem shape constants (hardcoded per contract).
B = 8          # batch == n_cores
C = 64         # channels (Cin == Hid == Cout == 64)
H = 128        # full-res H == W
HW = H * H     # 16384
HD = 64        # pooled H == W
N = HD * HD    # 4096 tokens
NB = 8         # n-blocks of 512 tokens
BLK = N // NB  # 512
MT = 32        # m-tiles of 128 tokens
NP = 4         # block pairs (1024 tokens each)
NEG_SLOPE = 0.2
BN_EPS = 1e-5

# fp8 scale plan: q8 = AQ*q_raw, k8 = AK*k_raw; stride-0 DoubleRow doubles
# the product; exp() folds sigma = attn_scale / (AQ*AK*2) back in.
AQ = 4.0
AK = 4.0
SIG = (C ** -0.5) / (AQ * AK * 2.0)   # 2^-8
SV = 4.0                              # V'8 = SV * Wo' V
SVC = 0.5625 / SV                     # stt scalar: 0.5625/SV

# exp engine split: each [128,1024] tile is split column-wise, ACT takes
# the first ECOLS columns, DVE (custom poly op) the rest — both engines
# run in lockstep on every tile.
ECOLS = 672


def build_program(debug=False, taps=False):
    """Build the per-core (SPMD) bass program. Returns (nc, io_names)."""
    nc = bacc.Bacc(
        "TRN2",
        target_bir_lowering=False,
        debug=debug,
        enable_asserts=False,
        num_devices=B,
    )

    # DRAM I/O (per-core slices of the batch; weights replicated).
    rgb_d = nc.dram_tensor("rgb", [C, HW], BF16, kind="ExternalInput").ap()
    freq_d = nc.dram_tensor("freq", [C, HW], FP8, kind="ExternalInput").ap()
    wq_d = nc.dram_tensor("wq_l", [C, C], BF16, kind="ExternalInput").ap()
    wk_d = nc.dram_tensor("wk_l", [C + 1, C], BF16, kind="ExternalInput").ap()
    wv_d = nc.dram_tensor("wv2", [C + 1, C], BF16, kind="ExternalInput").ap()
    wi_d = nc.dram_tensor("wi_l", [2 * C, C], FP8, kind="ExternalInput").ap()
    b75_d = nc.dram_tensor("b75", [C, 1], F32, kind="ExternalInput").ap()
    onesb_d = nc.dram_tensor("onesb", [1, N], BF16, kind="ExternalInput").ap()
    out_d = nc.dram_tensor("out", [C, HW], BF16, kind="ExternalOutput").ap()
    recd = nc.dram_tensor("rec_scratch", [NB, BLK], F32).ap()
    if taps:
        fds_o = nc.dram_tensor("fds_o", [C + 1, N], BF16, kind="ExternalOutput").ap()
        qd_o = nc.dram_tensor("qd_o", [C, N], FP8, kind="ExternalOutput").ap()
        kd_o = nc.dram_tensor("kd_o", [C, N], FP8, kind="ExternalOutput").ap()
        vt_o = nc.dram_tensor("vt_o", [128, MT * C], FP8,
                              kind="ExternalOutput").ap()
        av_o = nc.dram_tensor("av_o", [C + 1, N], F32, kind="ExternalOutput").ap()
        t56_o = nc.dram_tensor("t56_o", [C, N], BF16, kind="ExternalOutput").ap()
        xup_o = nc.dram_tensor("xup_o", [C, 2 * N], BF16, kind="ExternalOutput").ap()
        v_o = nc.dram_tensor("v_o", [C, HW], BF16, kind="ExternalOutput").ap()

    with tile.TileContext(nc) as tc:
        with (
            tc.tile_pool(name="const", bufs=1) as cpool,
            tc.tile_pool(name="persist", bufs=1) as perm,
        ):
            # ---- constants ----
            wq_t = cpool.tile([C, C], BF16, tag="wq")
            wk_t = cpool.tile([C + 1, C], BF16, tag="wk")
            wv_t = cpool.tile([C + 1, C], BF16, tag="wv")
            wi_t = cpool.tile([2 * C, C], FP8, tag="wi")
            b75_t = cpool.tile([C, 1], F32, tag="b75")
            nc.sync.dma_start(wq_t[:], wq_d)
            nc.sync.dma_start(wk_t[:], wk_d)
            nc.sync.dma_start(wv_t[:], wv_d)
            nc.sync.dma_start(wi_t[:], wi_d)
            nc.sync.dma_start(b75_t[:], b75_d)

            # ---- persistent SBUF tensors ----
            rgb_t = perm.tile([C, HW], BF16, tag="rgb")      # Q rhs + residual
            fds_t = perm.tile([C + 1, N], BF16, tag="fds")   # pooled freq +ones
            qd8_t = perm.tile([C, N], FP8, tag="qd8")        # q8 fp8
            kd8_t = perm.tile([C, N], FP8, tag="kd8")        # k8 fp8
            vt8_t = perm.tile([128, MT * C], FP8, tag="vt8")  # V'8^T tiles
            one8_t = perm.tile([128, 128], FP8, tag="one8")  # DR den-dup lhsT

            for p in range(NP):
                sl = slice(p * 4096, (p + 1) * 4096)
                nc.sync.dma_start(rgb_t[:, sl], rgb_d[:, sl])
            nc.gpsimd.dma_start(fds_t[C : C + 1, :], onesb_d)

            with (
                tc.tile_pool(name="p1sb", bufs=1) as p1sb,
                tc.tile_pool(name="pp1", bufs=2, space="PSUM") as pp1,
                tc.tile_pool(name="ppv", bufs=2, space="PSUM") as ppv,
            ):
                freq_t = p1sb.tile([C, HW], FP8, tag="freq")
                for p in range(NP):
                    sl = slice(p * 4096, (p + 1) * 4096)
                    nc.scalar.dma_start(freq_t[:, sl], freq_d[:, sl])

                # ---- phase 1a: pool freq via 4 accumulating fp8 matmuls ----
                freq_r = freq_t[:].rearrange(
                    "p (r a x c) -> p r a x c", r=HD, a=2, x=HD, c=2
                )
                for b in range(NB):
                    sl = slice(b * BLK, (b + 1) * BLK)
                    psf = pp1.tile([C, BLK], F32, tag="psf")
                    k = 0
                    for dy in range(2):
                        for dx in range(2):
                            nc.tensor.matmul(
                                psf[:],
                                wi_t[0:C, :],
                                freq_r[:, 8 * b : 8 * b + 8, dy, :, dx],
                                start=(k == 0),
                                stop=(k == 3),
                            )
                            k += 1
                    nc.vector.tensor_copy(fds_t[0:C, sl], psf[:])

                # ---- phase 1b: K (wk2 includes q-bias row as output 64) ----
                for b in range(NB):
                    sl = slice(b * BLK, (b + 1) * BLK)
                    psk = pp1.tile([C, BLK], F32, tag="psk")
                    nc.tensor.matmul(
                        psk[:], wk_t[:], fds_t[:, sl], start=True, stop=True
                    )
                    nc.scalar.copy(kd8_t[:, sl], psk[:])

                # ---- phase 1b2: Q (pool+AQ folded; bias via kd8 row 64) ----
                rgb_r = rgb_t[:].rearrange(
                    "p (r a x c) -> p r a x c", r=HD, a=2, x=HD, c=2
                )
                for b in range(NB):
                    sl = slice(b * BLK, (b + 1) * BLK)
                    psq = pp1.tile([C, BLK], F32, tag="psq")
                    k = 0
                    for dy in range(2):
                        for dx in range(2):
                            nc.tensor.matmul(
                                psq[:],
                                wq_t[:],
                                rgb_r[:, 8 * b : 8 * b + 8, dy, :, dx],
                                start=(k == 0),
                                stop=(k == 3),
                            )
                            k += 1
                    nc.vector.tensor_copy(qd8_t[:, sl], psq[:])

                # ---- phase 1c: V'8^T tiles (4 m-tiles per psum tile) ----
                nc.gpsimd.memset(one8_t[:], 1.0)
                for gv in range(8):
                    psv = ppv.tile([128, 4 * C], F32, tag="psv")
                    for j in range(4):
                        mt = 4 * gv + j
                        nc.tensor.matmul(
                            psv[:, j * C : (j + 1) * C],
                            fds_t[:, mt * 128 : (mt + 1) * 128],
                            wv_t[:],
                            start=True,
                            stop=True,
                        )
                    csl = slice(gv * 4 * C, (gv + 1) * 4 * C)
                    nc.scalar.copy(vt8_t[:, csl], psv[:])

            if taps:
                nc.sync.dma_start(fds_o, fds_t[:])
                nc.sync.dma_start(vt_o, vt8_t[:])

            # ---- phase 2: attention + epilogue ----
            with (
                tc.tile_pool(name="et", bufs=5) as etp,
                tc.tile_pool(name="epi1", bufs=1) as epi1,
                tc.tile_pool(name="epi2", bufs=2) as epi2,
                tc.tile_pool(name="fin", bufs=2) as fin,
                tc.tile_pool(name="otp", bufs=2) as otp,
                tc.tile_pool(name="ps2", bufs=3, space="PSUM") as ps2,
                tc.tile_pool(name="avp", bufs=1, space="PSUM") as avp,
            ):
                prev = {}
                av_tiles = {}
                pending = []   # software-pipelined AV stage
                AV_DELAY = 3

                def issue_av(item):
                    av, b, g, et = item
                    etv = et[:].rearrange("m (a n) -> m a n", a=2)
                    nc.tensor.matmul(
                        av[:, 0:BLK],
                        vt8_t[:, g * 2 * C : (g * 2 + 2) * C]
                        .rearrange("m (a c) -> m a c", a=2),
                        etv,
                        start=(g == 0),
                        stop=(g == 15),
                        perf_mode=DR,
                    )
                    # denominator, broadcast over 64 partitions by a ones
                    # lhsT — no DRAM bounce needed for the reciprocal
                    nc.tensor.matmul(
                        av[:, BLK : 2 * BLK],
                        one8_t[:].rearrange("m (a c) -> m a c", a=2),
                        etv,
                        start=(g == 0),
                        stop=(g == 15),
                        perf_mode=DR,
                    )
                    if g == 15:
                        stage_norm(b)
                        if b % 2 == 1:
                            epilogue(b // 2)

                t56_tiles = {}

                def stage_norm(b):
                    """Per-block reciprocal + normalize as soon as block b's
                    AV accumulation stops (frees the av psum quickly)."""
                    p = b // 2
                    if b % 2 == 0:
                        t56 = epi1.tile([C, 1024], BF16, tag="t56")
                        t56_tiles[p] = t56
                    t56 = t56_tiles[p]
                    av = av_tiles.pop(b)
                    rbs = epi1.tile([C, BLK], F32, tag="rbs")
                    nc.vector.reciprocal_approx_fast(
                        out=rbs[:], in_=av[:, BLK : 2 * BLK]
                    )
                    h = (b % 2) * BLK
                    nc.vector.scalar_tensor_tensor(
                        t56[:, h : h + BLK], av[:, 0:BLK], SVC, rbs[:],
                        ALU.mult, ALU.mult,
                    )
                    if taps:
                        nc.sync.dma_start(
                            av_o[:, b * BLK : (b + 1) * BLK], av[:, 0:BLK]
                        )

                def attn_block(b):
                    """QK + exp for block b; AV lags AV_DELAY tiles behind."""
                    av = avp.tile([C, 2 * BLK], F32, tag="av")
                    av_tiles[b] = av
                    nsl = slice(b * BLK, (b + 1) * BLK)
                    qv = (
                        qd8_t[:, nsl]
                        .rearrange("k (o n) -> k o n", o=1)
                        .to_broadcast((C, 2, BLK))
                    )
                    for g in range(16):
                        ps = ps2.tile([128, 1024], F32, tag="ps")
                        for j in range(2):
                            mt = 2 * g + j
                            kv = (
                                kd8_t[:, mt * 128 : (mt + 1) * 128]
                                .rearrange("k (o m) -> k o m", o=1)
                                .to_broadcast((C, 2, 128))
                            )
                            nc.tensor.matmul(
                                ps[:, j * BLK : (j + 1) * BLK],
                                kv,
                                qv,
                                start=True,
                                stop=True,
                                perf_mode=DR,
                            )
                        et = etp.tile([128, 1024], FP8, tag="et")
                        nc.scalar.activation(
                            et[:, 0:ECOLS], ps[:, 0:ECOLS], AF.Exp, scale=SIG
                        )
                        nc.vector._custom_dve(
                            EXP_OP, out=et[:, ECOLS:1024], in0=ps[:, ECOLS:1024],
                            s0=EC0, s1=EC1 * SIG, imm2=EC2 * SIG * SIG,
                        )
                        pending.append((av, b, g, et))
                        while len(pending) > AV_DELAY:
                            issue_av(pending.pop(0))

                def finalize(q):
                    """LReLU + residual + output DMA for pair q's v tile,
                    processed in two half-pair chunks so DVE/Pool/DMA
                    pipeline; the last pair keeps max on DVE for tail."""
                    pv = prev["v"]
                    tail = q == NP - 1
                    for h in range(2):
                        hsl = slice(h * 2048, (h + 1) * 2048)
                        l02 = fin.tile([C, 2048], BF16, tag="l02")
                        nc.vector.tensor_scalar(
                            l02[:], pv[:, hsl], NEG_SLOPE, None, ALU.mult
                        )
                        y4 = fin.tile([C, 2048], BF16, tag="y4")
                        nc.vector.tensor_tensor(
                            y4[:], pv[:, hsl], l02[:], ALU.max
                        )
                        osl = slice(q * 4096 + h * 2048, q * 4096 + (h + 1) * 2048)
                        ot = otp.tile([C, 2048], BF16, tag="ot")
                        otv = ot[:].rearrange(
                            "c (r x a) -> c r x a", r=16, x=HD, a=2
                        )
                        y4v = y4[:].rearrange(
                            "c (r a x) -> c r a x", r=16, a=2, x=HD
                        )
                        rgv = rgb_t[:, osl].rearrange(
                            "c (r x a) -> c r x a", r=16, x=HD, a=2
                        )
                        nc.gpsimd.tensor_tensor(
                            otv[:, :, :, 0], y4v[:, :, 0, :], rgv[:, :, :, 0],
                            ALU.add,
                        )
                        nc.gpsimd.tensor_tensor(
                            otv[:, :, :, 1], y4v[:, :, 1, :], rgv[:, :, :, 1],
                            ALU.add,
                        )
                        nc.sync.dma_start(out_d[:, osl], ot[:])
                    if taps:
                        nc.sync.dma_start(
                            v_o[:, q * 4096 : (q + 1) * 4096], pv[:]
                        )

                def epilogue(p):
                    b0, b1 = 2 * p, 2 * p + 1
                    t56 = t56_tiles.pop(p)
                    if taps:
                        nc.sync.dma_start(t56_o[:, b0 * BLK : (b1 + 1) * BLK],
                                          t56[:])
                    # p18 = t56/3 + 0.75*b'  (carries the conv/BN bias)
                    p18 = epi1.tile([C, 1024], BF16, tag="p18")
                    nc.vector.tensor_scalar(
                        p18[:], t56[:], 1.0 / 3.0, b75_t[:], ALU.mult, ALU.add
                    )

                    # x-upsample: xup75 = 0.75*(xup + b'), layout [r16, par2, x64]
                    xup = epi2.tile([C, 2048], BF16, tag="xup")
                    xv = xup[:].rearrange("c (r a x) -> c r a x", r=16, a=2, x=HD)
                    t56v = t56[:].rearrange("c (r x) -> c r x", r=16, x=HD)
                    p18v = p18[:].rearrange("c (r x) -> c r x", r=16, x=HD)
                    # even out col 2i: p18[i-1] + t56[i] (i>=1); i=0 clamps
                    nc.gpsimd.tensor_tensor(
                        xv[:, :, 0, 1:64], p18v[:, :, 0:63], t56v[:, :, 1:64],
                        ALU.add,
                    )
                    nc.gpsimd.tensor_tensor(
                        xv[:, :, 0, 0:1], p18v[:, :, 0:1], t56v[:, :, 0:1],
                        ALU.add,
                    )
                    # odd out col 2i+1: t56[i] + p18[i+1] (i<=62); i=63 clamps
                    nc.gpsimd.tensor_tensor(
                        xv[:, :, 1, 0:63], t56v[:, :, 0:63], p18v[:, :, 1:64],
                        ALU.add,
                    )
                    nc.gpsimd.tensor_tensor(
                        xv[:, :, 1, 63:64], t56v[:, :, 63:64], p18v[:, :, 63:64],
                        ALU.add,
                    )
                    if taps:
                        nc.sync.dma_start(
                            xup_o[:, b0 * 1024 : (b1 + 1) * 1024], xup[:]
                        )
                    # x18 = xup75/3
                    x18 = epi2.tile([C, 2048], BF16, tag="x18")
                    nc.vector.tensor_scalar(
                        x18[:], xup[:], 1.0 / 3.0, None, ALU.mult
                    )

                    # y-upsample rows: v[r'] layout [r'32, 128]
                    v = epi2.tile([C, 4096], BF16, tag="v")
                    vv = v[:].rearrange("c (r w) -> c r w", r=32, w=H)
                    xr = xup[:].rearrange("c (r w) -> c r w", r=16, w=H)
                    x18r = x18[:].rearrange("c (r w) -> c r w", r=16, w=H)
                    # even rows 2j = x18[j-1] + xup75[j], j=1..15
                    nc.gpsimd.tensor_tensor(
                        vv[:, 2:32:2, :], x18r[:, 0:15, :], xr[:, 1:16, :],
                        ALU.add,
                    )
                    # even row 0: boundary with previous pair (or clamp)
                    if p == 0:
                        nc.vector.tensor_tensor(
                            vv[:, 0:1, :], x18r[:, 0:1, :], xr[:, 0:1, :],
                            ALU.add,
                        )
                    else:
                        pxr18 = prev["x18"][:].rearrange(
                            "c (r w) -> c r w", r=16, w=H
                        )
                        nc.vector.tensor_tensor(
                            vv[:, 0:1, :], pxr18[:, 15:16, :], xr[:, 0:1, :],
                            ALU.add,
                        )
                        # previous pair's last row: xup75_prev[15] + x18[0]
                        pvv = prev["v"][:].rearrange("c (r w) -> c r w", r=32, w=H)
                        pxr = prev["xup"][:].rearrange(
                            "c (r w) -> c r w", r=16, w=H
                        )
                        nc.vector.tensor_tensor(
                            pvv[:, 31:32, :], pxr[:, 15:16, :], x18r[:, 0:1, :],
                            ALU.add,
                        )
                        finalize(p - 1)
                    # odd rows 2j+1 = xup75[j] + x18[j+1], j=0..14
                    nc.gpsimd.tensor_tensor(
                        vv[:, 1:31:2, :], xr[:, 0:15, :], x18r[:, 1:16, :],
                        ALU.add,
                    )
                    if p == NP - 1:
                        # last image row clamps: xup75[15] + x18[15]
                        nc.vector.tensor_tensor(
                            vv[:, 31:32, :], xr[:, 15:16, :], x18r[:, 15:16, :],
                            ALU.add,
                        )
                    prev.update(v=v, xup=xup, x18=x18)

                for b in range(NB):
                    attn_block(b)
                while pending:
                    issue_av(pending.pop(0))
                finalize(NP - 1)
                if taps:
                    nc.sync.dma_start(qd_o, qd8_t[:])
                    nc.sync.dma_start(kd_o, kd8_t[:])

    nc.compile()
    return nc, None


def _prep_weights(w_q, b_q, w_k, b_k, w_v, b_v, w_o, b_o, bn_gamma, bn_beta,
                  bn_mean, bn_var):
    bf = ml_dtypes.bfloat16
    f8 = ml_dtypes.float8_e4m3
    inv = bn_gamma / np.sqrt(bn_var + BN_EPS)
    wo_p = w_o * inv[:, None]                     # BN-folded conv weight
    bprime = inv * (b_o - bn_mean) + bn_beta      # BN-folded conv bias

    # Q: pool(0.25) and AQ folded; bias handled via the wk2 extra column
    wq_l = (w_q.T * (0.25 * AQ)).astype(bf)
    # K: AK folded (q-bias dropped: costs ~5e-5 rel err, saves the DR
    # contraction row)
    wk_l = (np.vstack([w_k.T, b_k[None, :]]) * AK).astype(bf)
    # V': wv2 = wv_l @ M folds conv into V and keeps the ones channel
    wv_l = np.zeros((C + 1, C + 1), np.float32)
    wv_l[0:C, 0:C] = w_v.T
    wv_l[C, 0:C] = b_v
    wv_l[C, C] = 1.0
    M = np.zeros((C + 1, C), np.float32)
    M[0:C, 0:C] = SV * wo_p.T
    wv2 = (wv_l @ M).astype(bf)
    eye = 0.25 * np.eye(C, dtype=np.float32)
    wi_l = np.vstack([eye, eye]).astype(f8)
    b75 = (0.75 * bprime)[:, None].astype(np.float32)
    return dict(wq_l=wq_l, wk_l=wk_l, wv2=wv2, wi_l=wi_l, b75=b75,
                onesb=np.ones((1, N), bf))


_CACHED = {}


def kernel(**inputs):
    bf = ml_dtypes.bfloat16
    f8 = ml_dtypes.float8_e4m3
    rgb = np.asarray(inputs["rgb"], np.float32)
    freq = np.asarray(inputs["freq"], np.float32)
    wts = _prep_weights(
        np.asarray(inputs["w_q"], np.float32), np.asarray(inputs["b_q"], np.float32),
        np.asarray(inputs["w_k"], np.float32), np.asarray(inputs["b_k"], np.float32),
        np.asarray(inputs["w_v"], np.float32), np.asarray(inputs["b_v"], np.float32),
        np.asarray(inputs["w_o"], np.float32), np.asarray(inputs["b_o"], np.float32),
        np.asarray(inputs["bn_gamma"], np.float32),
        np.asarray(inputs["bn_beta"], np.float32),
        np.asarray(inputs["bn_mean"], np.float32),
        np.asarray(inputs["bn_var"], np.float32),
    )
    if "nc" not in _CACHED:
        _CACHED["nc"], _ = build_program()
    nc = _CACHED["nc"]
    in_maps = []
    for i in range(B):
        m = dict(wts)
        m["rgb"] = np.ascontiguousarray(rgb[i].reshape(C, HW)).astype(bf)
        m["freq"] = np.ascontiguousarray(freq[i].reshape(C, HW)).astype(f8)
        in_maps.append(m)
    res = run_bass_kernel_spmd(nc, in_maps, list(range(B)))
    out = np.stack([res.results[i]["out"] for i in range(B)])
    return out.reshape(B, C, H, H).astype(np.float32)


if __name__ == "__main__":
    nc, _ = build_program()
    print("program built OK")
